# revision 1
# baseline (speedup 1.0000x reference)
"""Bottleneck adapter (LayerNorm -> down-proj -> GELU -> up-proj -> residual)
as a Bass/Tile kernel for Trainium2, data-parallel over 8 NeuronCores.

Math (per token t, d_model D=2048, rank R=32):
    mu    = mean(x_t);  var = mean(x_t^2) - mu^2;  rstd = 1/sqrt(var+eps)
    down  = ln(x_t) @ w_down + b_down
          = rstd * (x_t @ W - mu * S) + b2        # W = gamma[:,None]*w_down
                                                  # S = colsum(W), b2 = beta@w_down + b_down
    out_t = x_t + gelu(down) @ w_up + b_up

Implementation notes (from cost-model + HW slope measurements):
  - down/up matmuls in bf16 (fp32 matmul is quarter-rate on the PE); the
    residual path and all statistics stay fp32.
  - x is PE-transposed per 128x128 chunk (transpose-mode matmul), batched
    4-chunks-per-PSUM-bank, copied to SBUF with a bf16 cast split across
    ACT and DVE.
  - The down matmul keeps TOKENS on the output partition axis
    (lhsT = xT chunk, rhs = [W | ones]), so mean (the ones-column) and all
    LN statistics are per-partition scalars: the correction needs no
    partition-broadcast at all, just tensor_scalar/scalar_tensor_tensor.
  - rstd = rsqrt(var+eps) on DVE with the int-bit-trick seed (0x5f3759df)
    + 2 Newton iterations -- avoids the ACT Sqrt function-table load that
    would thrash against Gelu every tile.
  - Gelu (exact, erf-based LUT) is the only table-based ACT function used,
    so its table loads exactly once. Copy/Square/Identity are in every set.
  - b_up rides as a 33rd contraction row of the up matmul against constant
    ones rows in persistent gelu-output tiles.
  - 4-stage software pipeline (front / -- / mid / back) keeps the PE fed
    with tile i's transposes+matmuls while tile i-2's stats chain crosses
    engines; x loads ride the SP HWDGE ring, stores the ACT ring.
"""

import numpy as np

import concourse.bacc as bacc
import concourse.bass as bass
import concourse.tile as tile
from concourse import mybir

F32 = mybir.dt.float32
BF16 = mybir.dt.bfloat16
I32 = mybir.dt.int32
AF = mybir.ActivationFunctionType
ALU = mybir.AluOpType

D = 2048          # d_model
R = 32            # adapter rank
N_CORES = 8
TOK_TOTAL = 4 * 4096
TOK_PER_CORE = TOK_TOTAL // N_CORES   # 2048
P = 128           # partitions / tokens per tile
N_TILES = TOK_PER_CORE // P           # 16
N_CHUNK = D // P                      # 16 chunks of d per tile
LN_EPS = 1e-5
UP_N = 512        # free-dim per up matmul (one PSUM bank)
N_UP = D // UP_N  # 4
XB = 4            # transpose chunks batched per PSUM bank ([128, 512])
MAGIC = 0x5F3759DF  # rsqrt seed


def build_program(reps=1):
    """reps>1 repeats the whole computation in one NEFF — used only by the
    timing harness (wall-clock slope over reps isolates on-device time)."""
    nc = bacc.Bacc(
        "TRN2",
        target_bir_lowering=False,
        debug=False,
        num_devices=N_CORES,
    )

    x_d = nc.dram_tensor("x", [TOK_PER_CORE, D], F32, kind="ExternalInput").ap()
    w_d = nc.dram_tensor("wc", [P, N_CHUNK, R + 1], BF16, kind="ExternalInput").ap()
    wu_d = nc.dram_tensor("wu", [R + 1, D], BF16, kind="ExternalInput").ap()
    sd_d = nc.dram_tensor("sd_bc", [P, R], F32, kind="ExternalInput").ap()
    b2_d = nc.dram_tensor("b2_bc", [P, R], F32, kind="ExternalInput").ap()
    id_d = nc.dram_tensor("ident", [P, P], F32, kind="ExternalInput").ap()
    out_d = nc.dram_tensor("out", [TOK_PER_CORE, D], F32, kind="ExternalOutput").ap()

    with tile.TileContext(nc) as tc:
        with (
            tc.tile_pool(name="consts", bufs=1) as cpool,
            tc.tile_pool(name="xin", bufs=6) as xpool,
            tc.tile_pool(name="sq", bufs=2) as sqpool,
            tc.tile_pool(name="ssqp", bufs=4) as ssqpool,
            tc.tile_pool(name="xt", bufs=3) as xtpool,
            tc.tile_pool(name="outs", bufs=2) as opool,
            tc.tile_pool(name="small", bufs=2) as spool,
            tc.tile_pool(name="ps_xt", bufs=2, space="PSUM") as ps_xt,
            tc.tile_pool(name="ps_dn", bufs=3, space="PSUM") as ps_dn,
            tc.tile_pool(name="ps_gt", bufs=2, space="PSUM") as ps_gt,
            tc.tile_pool(name="ps_up", bufs=1, space="PSUM") as ps_up,
        ):
            # ---- one-time constant loads / setup ----
            w_sb = cpool.tile([P, N_CHUNK, R + 1], BF16)  # [W | ones] chunks
            nc.sync.dma_start(w_sb[:], w_d[:])
            wu_sb = cpool.tile([R + 1, D], BF16)          # [w_up; b_up]
            nc.sync.dma_start(wu_sb[:], wu_d[:])
            sd_sb = cpool.tile([P, R], F32)               # colsum(W)/D, bcast
            nc.sync.dma_start(sd_sb[:], sd_d[:])
            b2_sb = cpool.tile([P, R], F32)               # beta@w_down+b_down, bcast
            nc.sync.dma_start(b2_sb[:], b2_d[:])
            id_sb = cpool.tile([P, P], F32)               # identity for PE transpose
            nc.sync.dma_start(id_sb[:], id_d[:])
            magic_sb = cpool.tile([P, 1], I32)            # rsqrt seed constant
            nc.vector.memset(magic_sb[:], MAGIC)
            # persistent gelu-output tiles; row R is the ones-row for b_up
            gts = [cpool.tile([R + 1, P], BF16, tag=f"gt{j}", name=f"gt{j}")
                   for j in range(3)]
            for g in gts:
                nc.vector.memset(g[R:R + 1, :], 1.0)

            # Per-tile state passed between pipeline stages
            state = {}

            def stage_front(i):
                """Load + sumsq + PE transposes + bf16 down matmuls."""
                ti = i % N_TILES
                tok = slice(ti * P, (ti + 1) * P)
                x_t = xpool.tile([P, D], F32, tag="x", name=f"x_{i}")
                nc.sync.dma_start(x_t[:], x_d[tok, :])

                sq_scr = sqpool.tile([P, D], F32, tag="scr", name=f"sq_{i}")
                ssq = ssqpool.tile([P, 1], F32, tag="ssq", name=f"ssq_{i}")
                nc.scalar.activation(sq_scr[:], x_t[:], AF.Square, accum_out=ssq[:])

                xt_sb = xtpool.tile([P, D], BF16, tag="xt", name=f"xt_{i}")
                for b in range(N_CHUNK // XB):
                    xt_ps = ps_xt.tile([P, XB * P], F32, tag="xtps",
                                       name=f"xtps_{i}_{b}")
                    for c in range(XB):
                        nc.tensor.transpose(
                            xt_ps[:, c * P:(c + 1) * P],
                            x_t[:, (b * XB + c) * P:(b * XB + c + 1) * P],
                            id_sb[:],
                        )
                    dst = xt_sb[:, b * XB * P:(b + 1) * XB * P]
                    if b % 2 == 0:
                        nc.scalar.copy(dst, xt_ps[:])         # ACT, casts to bf16
                    else:
                        nc.vector.tensor_copy(dst, xt_ps[:])  # DVE, casts to bf16

                # down-proj, tokens on partitions: xT_c^T @ [W_c | 1]
                dn_ps = ps_dn.tile([P, R + 1], F32, tag="dn", name=f"dn_{i}")
                for c in range(N_CHUNK):
                    nc.tensor.matmul(
                        dn_ps[:], xt_sb[:, c * P:(c + 1) * P], w_sb[:, c, :],
                        start=(c == 0), stop=(c == N_CHUNK - 1),
                    )
                state[i] = {"x_t": x_t, "ssq": ssq, "dn_ps": dn_ps}

            def stage_mid(i):
                """LN stats -> rstd (Newton, DVE) -> correction -> GELU -> g^T."""
                st = state[i]
                ssq, dn_ps = st["ssq"], st["dn_ps"]
                s1 = dn_ps[:, R:R + 1]                      # sum_d x  (= D*mu)

                # var = (ssq - s1^2/D)/D ; all per-partition [128,1] f32
                # (s1 lives in PSUM; DVE has a single PSUM read port, so pull
                # it into SBUF before squaring it)
                s1_sb = spool.tile([P, 1], F32, tag="s1", name=f"s1_{i}")
                nc.vector.tensor_scalar(s1_sb[:], s1, 1.0, None, ALU.mult)
                p_t = spool.tile([P, 1], F32, tag="p", name=f"p_{i}")
                nc.vector.tensor_mul(p_t[:], s1_sb[:], s1_sb[:])
                v = spool.tile([P, 1], F32, tag="v", name=f"v_{i}")
                nc.vector.scalar_tensor_tensor(v[:], p_t[:], -1.0 / D, ssq[:],
                                               ALU.mult, ALU.add)
                nc.vector.tensor_scalar(v[:], v[:], 1.0 / D, LN_EPS,
                                        ALU.mult, ALU.add)
                # rstd = rsqrt(v): bit-trick seed + 2 Newton iterations
                yi = spool.tile([P, 1], I32, tag="yi", name=f"yi_{i}")
                nc.vector.tensor_scalar(yi[:], v[:].bitcast(I32), 1, None,
                                        ALU.logical_shift_right)
                nc.vector.tensor_sub(yi[:], magic_sb[:], yi[:])
                y = yi[:].bitcast(F32)
                rstd = spool.tile([P, 1], F32, tag="rstd", name=f"rstd_{i}")
                t1 = spool.tile([P, 1], F32, tag="nt1", name=f"nt1_{i}")
                for it_n in range(2):
                    nc.vector.tensor_mul(t1[:], y, y)
                    nc.vector.tensor_mul(t1[:], t1[:], v[:])
                    nc.vector.tensor_scalar(t1[:], t1[:], -0.5, 1.5,
                                            ALU.mult, ALU.add)
                    if it_n == 0:
                        nc.vector.tensor_mul(yi[:].bitcast(F32), y, t1[:])
                    else:
                        nc.vector.tensor_mul(rstd[:], y, t1[:])
                mrs = spool.tile([P, 1], F32, tag="mrs", name=f"mrs_{i}")
                nc.vector.tensor_mul(mrs[:], s1_sb[:], rstd[:])   # = D*mu*rstd

                # o2 = (S/D)*mrs - b2 ; gin = rstd*down_raw - o2
                o2 = spool.tile([P, R], F32, tag="o2", name=f"o2_{i}")
                nc.vector.scalar_tensor_tensor(o2[:], sd_sb[:], mrs[:], b2_sb[:],
                                               ALU.mult, ALU.subtract)
                gin = spool.tile([P, R], F32, tag="gin", name=f"gin_{i}")
                nc.vector.scalar_tensor_tensor(gin[:], dn_ps[:, 0:R], rstd[:],
                                               o2[:], ALU.mult, ALU.subtract)

                # exact GELU, then transpose g -> [R, 128] for the up matmul
                g_t = spool.tile([P, R], F32, tag="g", name=f"g_{i}")
                nc.scalar.activation(g_t[:], gin[:], AF.Gelu)
                gt_ps = ps_gt.tile([R, P], F32, tag="gt", name=f"gtps_{i}")
                nc.tensor.transpose(gt_ps[:], g_t[:], id_sb[:])
                nc.scalar.copy(gts[i % 3][0:R, :], gt_ps[:])  # casts to bf16

            def stage_back(i):
                """bf16 up-proj + residual + store."""
                ti = i % N_TILES
                tok = slice(ti * P, (ti + 1) * P)
                x_t = state[i]["x_t"]
                gt_sb = gts[i % 3]
                out_t = opool.tile([P, D], F32, tag="out", name=f"out_{i}")
                for j in range(N_UP):
                    js = slice(j * UP_N, (j + 1) * UP_N)
                    up_ps = ps_up.tile([P, UP_N], F32, tag="up", name=f"up_{i}_{j}")
                    nc.tensor.matmul(up_ps[:], gt_sb[:], wu_sb[:, js],
                                     start=True, stop=True)
                    nc.vector.tensor_add(out_t[:, js], x_t[:, js], up_ps[:])
                nc.scalar.dma_start(out_d[tok, :], out_t[:])
                del state[i]

            # 4-stage software pipeline: F(i) | - | M(i-2) | B(i-3).
            n_it = N_TILES * reps
            for it in range(n_it + 3):
                if it < n_it:
                    stage_front(it)
                if 0 <= it - 2 < n_it:
                    stage_mid(it - 2)
                if 0 <= it - 3 < n_it:
                    stage_back(it - 3)

    nc.compile()
    return nc


def make_param_maps(gamma, beta, w_down, b_down, w_up, b_up):
    import ml_dtypes

    f32 = np.float32
    bf16 = ml_dtypes.bfloat16
    gamma = np.asarray(gamma, f32)
    beta = np.asarray(beta, f32)
    w_down = np.asarray(w_down, f32)
    b_down = np.asarray(b_down, f32)
    w_up = np.asarray(w_up, f32)
    b_up = np.asarray(b_up, f32)

    W = (gamma[:, None] * w_down).astype(f32)                    # [D, R]
    W_bf = W.astype(bf16)
    w_aug = np.concatenate([W_bf, np.ones((D, 1), bf16)], axis=1)  # [D, R+1]
    wc = np.ascontiguousarray(
        w_aug.reshape(N_CHUNK, P, R + 1).transpose(1, 0, 2))       # [P, c, R+1]
    # S must match the bf16 W actually used in the matmul; fold in the 1/D
    S = W_bf.astype(f32).sum(axis=0)
    sd_bc = np.tile((S / D).astype(f32)[None, :], (P, 1))
    b2 = (beta @ w_down + b_down).astype(f32)
    b2_bc = np.tile(b2[None, :], (P, 1))
    wu = np.concatenate([w_up, b_up[None, :]], axis=0).astype(bf16)  # [R+1, D]
    ident = np.eye(P, dtype=f32)
    return {
        "wc": wc, "wu": wu, "sd_bc": sd_bc, "b2_bc": b2_bc, "ident": ident,
    }


_NC_CACHE = None


def _get_nc():
    global _NC_CACHE
    if _NC_CACHE is None:
        _NC_CACHE = build_program()
    return _NC_CACHE


LAST_RESULTS = None  # BassKernelResults from the most recent run (for test.py)


def kernel(x, gamma, beta, w_down, b_down, w_up, b_up):
    global LAST_RESULTS
    from concourse.bass_utils import run_bass_kernel_spmd

    x = np.asarray(x, np.float32)
    params = make_param_maps(gamma, beta, w_down, b_down, w_up, b_up)

    x_flat = x.reshape(TOK_TOTAL, D)
    in_maps = []
    for c in range(N_CORES):
        shard = np.ascontiguousarray(
            x_flat[c * TOK_PER_CORE:(c + 1) * TOK_PER_CORE]
        )
        in_maps.append({"x": shard, **params})

    nc = _get_nc()
    res = run_bass_kernel_spmd(nc, in_maps, list(range(N_CORES)))
    LAST_RESULTS = res
    out = np.concatenate([res.results[c]["out"] for c in range(N_CORES)], axis=0)
    return out.reshape(x.shape).astype(np.float32)

